# revision 16
# baseline (speedup 1.0000x reference)
"""Trainium2 Bass kernel for nn_MessagePassing (gnn_message_passing) — v4.

Math (per batch b = core b):
    coef[s,e] = sum_o adj[s,o] * edge[s,o,e]
    v[s,e,i]  = sum_j W[e,i,j] * node[s,j]
    out[s,i]  = sum_e coef[s,e] * v[s,e,i]

Design (batch-parallel over 8 cores; DVE-rate driven):
  * One SWDGE ring saturates the 16 DMA engines (~360GB/s read-side,
    ~91us for the 36MB). Ring order: t0 in halves up front, t1-t4 full,
    then first halves of t5/t6/t7, then their second halves — chunk
    sizes shrink toward the tail so post-drain DVE work is one half.
  * DVE runs ONLY the per-e tensor_tensor_reduce mul-accumulates
    (~84us, the binding resource), the 16 transpose copies in its
    known t0->t1 fill gap, and the final t7 chain. Chunked coefs
    accumulate via TTR's initial-value seeding — no merge adds.
  * ScalarE: all de-interleaves (full tiles split by int32-plane so
    STT e0-3 start after half the deint) + all scale ops + out copies.
  * PE: transposes, bf16 v-matmuls, and the identity-matmul e-reduction
    for tiles 0-6 (runs parallel to DVE's endgame).
  * t7 reduces via a short DVE chain over v7 parked early in SBUF.
"""

import numpy as np
from contextlib import ExitStack

import concourse.bass as bass
import concourse.bacc as bacc
import concourse.mybir as mybir
import concourse.tile as tile
from concourse.bass_utils import run_bass_kernel_spmd
from concourse.masks import make_identity

B, N, D, E = 8, 1024, 128, 8
P = 128
NT = N // P  # 8 s-tiles per core
H = N // 2

F32 = mybir.dt.float32
BF16 = mybir.dt.bfloat16
I32 = mybir.dt.int32
MUL = mybir.AluOpType.mult
ADD = mybir.AluOpType.add
LAST = NT - 1


def build_nc():
    nc = bacc.Bacc("TRN2", target_bir_lowering=False, debug=False, num_devices=B)

    node_d = nc.dram_tensor("node_state", [N, D], F32, kind="ExternalInput").ap()
    edge_d = nc.dram_tensor("edge_type_mat", [N, N, E], F32, kind="ExternalInput").ap()
    adj_d = nc.dram_tensor("adj_mat", [N, N], F32, kind="ExternalInput").ap()
    w_d = nc.dram_tensor("W", [E, D, D], F32, kind="ExternalInput").ap()
    out_d = nc.dram_tensor("out", [N, D], F32, kind="ExternalOutput").ap()

    with tile.TileContext(nc) as tc, ExitStack() as ctx:
        const_pool = ctx.enter_context(tc.tile_pool(name="const", bufs=1))
        edge_pool = ctx.enter_context(tc.tile_pool(name="edge", bufs=8))
        quad_pool = ctx.enter_context(tc.tile_pool(name="quad", bufs=2))
        work_pool = ctx.enter_context(tc.tile_pool(name="work", bufs=2))
        coef_pool = ctx.enter_context(tc.tile_pool(name="coefp", bufs=3))
        psv_pool = ctx.enter_context(tc.tile_pool(name="psv", bufs=6, space="PSUM"))
        pss_pool = ctx.enter_context(tc.tile_pool(name="pss", bufs=2, space="PSUM"))

        adj_r = adj_d.rearrange("(t p) o -> p t o", p=P)
        adj_tiles = [
            const_pool.tile([P, N], BF16, name=f"adj{t}") for t in range(NT)
        ]
        node_all = const_pool.tile([P, NT, D], F32)
        w_all = const_pool.tile([P, E, D], F32)  # [i, e, j]
        edge_tiles = [
            edge_pool.tile([P, N, E], BF16, tag="edge_t", name=f"et{t}")
            for t in range(NT)
        ]

        def dma_edge(t, lo, hi):
            nc.gpsimd.dma_start(
                edge_tiles[t][:, lo:hi, :], edge_d[t * P : t * P + P, lo:hi]
            )

        def dma_adj(t):
            nc.gpsimd.dma_start(adj_tiles[t][:], adj_r[:, t, :])

        # ---- ring: t0 halves | t1..t4 full | 5a 6a 7a | 5b 6b 7b ----
        dma_edge(0, 0, H)
        dma_adj(0)
        dma_edge(0, H, N)

        # ident early on the GpSimd queue, wire already started
        ident = const_pool.tile([P, P], F32)
        make_identity(nc, ident[:])
        ident_bf = const_pool.tile([P, P], BF16)
        nc.vector.tensor_copy(ident_bf[:], ident[:])

        # node/W on the idle sync ring (needed ~25us in; starvation OK)
        nc.sync.dma_start(node_all[:], node_d.rearrange("(t p) j -> p t j", p=P))
        nc.sync.dma_start(w_all[:], w_d.rearrange("e i j -> i e j"))

        for t in range(1, 5):
            dma_adj(t)
            dma_edge(t, 0, N)
        for t in (5, 6, LAST):
            dma_adj(t)
            dma_edge(t, 0, H)
        dma_edge(5, H, N)
        dma_edge(6, H, N)
        dma_edge(LAST, H, N)

        # node^T / W^T via PE transpose -> bf16 (copies run on DVE in its
        # t0->t1 fill gap; emitted later so they can't HOL-block the STTs)
        nodeT = const_pool.tile([P, N], BF16)
        wT = const_pool.tile([P, E, D], BF16)  # [j, e, i]
        tcopies = []
        srcs = [(node_all[:, 0, :], nodeT[:, 0:P])]
        srcs += [(w_all[:, e, :], wT[:, e, :]) for e in range(E)]
        srcs += [(node_all[:, t, :], nodeT[:, bass.ts(t, P)]) for t in range(1, NT)]
        for src_ap, dst_ap in srcs:
            pt = pss_pool.tile([P, P], F32, tag="ps_small")
            nc.tensor.transpose(pt[:], src_ap, ident[:])
            tcopies.append((dst_ap, pt))

        coef5 = const_pool.tile([P, E], F32)
        coef6 = const_pool.tile([P, E], F32)
        coef7 = const_pool.tile([P, E], F32)
        v7sb = const_pool.tile([P, E, D], BF16)

        chunk_quads = {}

        def deint(t, lo, hi, planes):
            """Sc de-interleave of edge[t][:, lo:hi] into the quad layout.
            Each chunk gets its own pool slot (lives only deint -> TTRs).
            planes=2 emits one op per int32-plane so the first 4 e-channels
            unblock after half the work."""
            quad = quad_pool.tile(
                [P, 2, N, 4], BF16, tag="quad", name=f"q{t}_{lo}"
            )
            chunk_quads[(t, lo)] = quad
            src = edge_tiles[t][:, lo:hi, :].bitcast(I32)
            for q in range(2) if planes == 2 else (None,):
                if q is None:
                    nc.scalar.copy(
                        quad[:, :, lo:hi, :].bitcast(I32),
                        src.rearrange("p n (q t) -> p q n t", q=2),
                    )
                else:
                    nc.scalar.copy(
                        quad[:, q, lo:hi, :].bitcast(I32),
                        src[:, :, 2 * q : 2 * q + 2],
                    )

        def ttr8(t, lo, hi, coef_dst, seed):
            """8 per-e STT mul-accumulates on DVE; seed=None starts the
            coef, otherwise a follow-up chunk lands in a temp slot that is
            merged with one [P,E] tensor_add."""
            quad = chunk_quads.pop((t, lo))
            scratch = work_pool.tile([P, N], BF16, tag="scratch")
            if seed is None:
                dst = coef_dst
            else:
                dst = work_pool.tile([P, E], F32, tag="coef_b", name=f"cb{t}{lo}")
            for e in range(E):
                q, j = divmod(e, 4)
                nc.vector.scalar_tensor_tensor(
                    out=scratch[:, 0 : hi - lo],
                    in0=quad[:, q, lo:hi, j],
                    scalar=1.0,
                    in1=adj_tiles[t][:, lo:hi],
                    op0=MUL,
                    op1=MUL,
                    accum_out=dst[:, e : e + 1],
                )
            if seed is not None:
                nc.vector.tensor_add(coef_dst[:], coef_dst[:], dst[:])

        def v_matmuls(t):
            psums = []
            for g in range(E // 4):
                pv = psv_pool.tile([P, 4, D], F32, tag="psum_v")
                nc.tensor.matmul(
                    pv[:],
                    lhsT=nodeT[:, bass.ts(t, P)],
                    rhs=wT[:, g * 4 : (g + 1) * 4, :],
                    start=True,
                    stop=True,
                )
                psums.append(pv)
            return psums

        state = {}

        def stage_reduce(t):
            """Sc scales + PE identity-matmul e-reduction + Sc out copy."""
            coef, psums = state.pop(t)
            sv = work_pool.tile([P, E, D], BF16, tag="sv")
            for e in range(E):
                nc.scalar.mul(
                    sv[:, e, :], psums[e // 4][:, e % 4, :], coef[:, e : e + 1]
                )
            acc = pss_pool.tile([P, D], F32, tag="ps_small")
            for e in range(E):
                nc.tensor.matmul(
                    acc[:],
                    lhsT=ident_bf[:],
                    rhs=sv[:, e, :],
                    start=(e == 0),
                    stop=(e == E - 1),
                )
            out_sb = work_pool.tile([P, D], F32, tag="out_sb")
            nc.scalar.copy(out_sb[:], acc[:])
            nc.sync.dma_start(out_d[bass.ts(t, P)], out_sb[:])

        # ---- software pipeline ----
        # t0 halves (deint is a single op per half: small chunks)
        coef0 = coef_pool.tile([P, E], F32, tag="coef")
        deint(0, 0, H, planes=1)
        ttr8(0, 0, H, coef0, seed=None)
        deint(0, H, N, planes=1)
        ttr8(0, H, N, coef0, seed=True)
        # DVE fill gap: the 16 transpose copies + park v7 in SBUF
        for dst_ap, pt in tcopies:
            nc.vector.tensor_copy(dst_ap, pt[:])
        psums7e = v_matmuls(LAST)
        for g in range(E // 4):
            nc.scalar.copy(v7sb[:, g * 4 : (g + 1) * 4, :], psums7e[g][:])
        state[0] = (coef0, v_matmuls(0))

        for t in range(1, 5):
            coef = coef_pool.tile([P, E], F32, tag="coef")
            deint(t, 0, N, planes=2)
            ttr8(t, 0, N, coef, seed=None)
            state[t] = (coef, v_matmuls(t))
            if t >= 2:
                stage_reduce(t - 2)

        # first halves of 5/6/7
        deint(5, 0, H, planes=1)
        ttr8(5, 0, H, coef5, seed=None)
        state[5] = (coef5, v_matmuls(5))
        deint(6, 0, H, planes=1)
        ttr8(6, 0, H, coef6, seed=None)
        state[6] = (coef6, v_matmuls(6))
        stage_reduce(3)
        deint(LAST, 0, H, planes=1)
        ttr8(LAST, 0, H, coef7, seed=None)

        # second halves; each seeds on the tile's existing coef
        deint(5, H, N, planes=1)
        ttr8(5, H, N, coef5, seed=True)
        stage_reduce(4)
        deint(6, H, N, planes=1)
        ttr8(6, H, N, coef6, seed=True)
        # d7b ahead of the r5 scales on Sc so the final deint is not
        # head-of-line blocked behind them (its data lands ~101us)
        deint(LAST, H, N, planes=1)
        stage_reduce(5)
        ttr8(LAST, H, N, coef7, seed=True)
        stage_reduce(6)

        # t7 closeout: short DVE chain over v7 (SBUF reads)
        bufs = [
            work_pool.tile([P, D], F32, tag="acc_a", name="chain_a"),
            work_pool.tile([P, D], F32, tag="acc_b", name="chain_b"),
        ]
        nc.vector.tensor_scalar_mul(bufs[0][:], v7sb[:, 0, :], coef7[:, 0:1])
        cur_ap = bufs[0][:]
        for e in range(1, E):
            nxt = bufs[e % 2]
            nc.vector.scalar_tensor_tensor(
                out=nxt[:],
                in0=v7sb[:, e, :],
                scalar=coef7[:, e : e + 1],
                in1=cur_ap,
                op0=MUL,
                op1=ADD,
            )
            cur_ap = nxt[:]
        nc.sync.dma_start(out_d[bass.ts(LAST, P)], cur_ap)

    nc.compile()
    return nc


_NC_CACHE = None


def get_nc():
    global _NC_CACHE
    if _NC_CACHE is None:
        _NC_CACHE = build_nc()
    return _NC_CACHE


def make_in_maps(node_state, edge_type_mat, adj_mat, W):
    return [
        {
            "node_state": np.ascontiguousarray(node_state[b], dtype=np.float32),
            "edge_type_mat": np.ascontiguousarray(edge_type_mat[b], dtype=np.float32),
            "adj_mat": np.ascontiguousarray(adj_mat[b], dtype=np.float32),
            "W": np.ascontiguousarray(W, dtype=np.float32),
        }
        for b in range(B)
    ]


def kernel(node_state, edge_type_mat, adj_mat, W):
    nc = get_nc()
    in_maps = make_in_maps(node_state, edge_type_mat, adj_mat, W)
    res = run_bass_kernel_spmd(nc, in_maps, list(range(B)))
    return np.stack([res.results[b]["out"] for b in range(B)], axis=0)


# revision 19
# speedup vs baseline: 1.1437x; 1.1437x over previous
"""Trainium2 Bass kernel for nn_MessagePassing (gnn_message_passing) — v4.

Math (per batch b = core b):
    coef[s,e] = sum_o adj[s,o] * edge[s,o,e]
    v[s,e,i]  = sum_j W[e,i,j] * node[s,j]
    out[s,i]  = sum_e coef[s,e] * v[s,e,i]

Design (batch-parallel over 8 cores; DVE-rate driven):
  * One SWDGE ring saturates the 16 DMA engines (~360GB/s read-side,
    ~91us for the 36MB). Ring order: t0 in halves up front, t1-t4 full,
    then first halves of t5/t6/t7, then their second halves — chunk
    sizes shrink toward the tail so post-drain DVE work is one half.
  * DVE runs ONLY the per-e tensor_tensor_reduce mul-accumulates
    (~84us, the binding resource), the 16 transpose copies in its
    known t0->t1 fill gap, and the final t7 chain. Chunked coefs
    accumulate via TTR's initial-value seeding — no merge adds.
  * ScalarE: all de-interleaves (full tiles split by int32-plane so
    STT e0-3 start after half the deint) + all scale ops + out copies.
  * PE: transposes, bf16 v-matmuls, and the identity-matmul e-reduction
    for tiles 0-6 (runs parallel to DVE's endgame).
  * t7 reduces via a short DVE chain over v7 parked early in SBUF.
"""

import numpy as np
from contextlib import ExitStack

import concourse.bass as bass
import concourse.bacc as bacc
import concourse.mybir as mybir
import concourse.tile as tile
from concourse.bass_utils import run_bass_kernel_spmd
from concourse.masks import make_identity

B, N, D, E = 8, 1024, 128, 8
P = 128
NT = N // P  # 8 s-tiles per core
H = N // 2

F32 = mybir.dt.float32
BF16 = mybir.dt.bfloat16
I32 = mybir.dt.int32
MUL = mybir.AluOpType.mult
ADD = mybir.AluOpType.add
LAST = NT - 1


def build_nc():
    nc = bacc.Bacc("TRN2", target_bir_lowering=False, debug=False, num_devices=B)

    node_d = nc.dram_tensor("node_state", [N, D], F32, kind="ExternalInput").ap()
    edge_d = nc.dram_tensor("edge_type_mat", [N, N, E], F32, kind="ExternalInput").ap()
    adj_d = nc.dram_tensor("adj_mat", [N, N], F32, kind="ExternalInput").ap()
    w_d = nc.dram_tensor("W", [E, D, D], F32, kind="ExternalInput").ap()
    out_d = nc.dram_tensor("out", [N, D], F32, kind="ExternalOutput").ap()

    with tile.TileContext(nc) as tc, ExitStack() as ctx:
        const_pool = ctx.enter_context(tc.tile_pool(name="const", bufs=1))
        edge_pool = ctx.enter_context(tc.tile_pool(name="edge", bufs=8))
        quad_pool = ctx.enter_context(tc.tile_pool(name="quad", bufs=2))
        work_pool = ctx.enter_context(tc.tile_pool(name="work", bufs=2))
        coef_pool = ctx.enter_context(tc.tile_pool(name="coefp", bufs=3))
        psv_pool = ctx.enter_context(tc.tile_pool(name="psv", bufs=6, space="PSUM"))
        pss_pool = ctx.enter_context(tc.tile_pool(name="pss", bufs=2, space="PSUM"))

        adj_r = adj_d.rearrange("(t p) o -> p t o", p=P)
        adj_tiles = [
            const_pool.tile([P, N], BF16, name=f"adj{t}") for t in range(NT)
        ]
        node_all = const_pool.tile([P, NT, D], F32)
        w_all = const_pool.tile([P, E, D], F32)  # [i, e, j]
        edge_tiles = [
            edge_pool.tile([P, N, E], BF16, tag="edge_t", name=f"et{t}")
            for t in range(NT)
        ]

        def dma_edge(t, lo, hi):
            nc.gpsimd.dma_start(
                edge_tiles[t][:, lo:hi, :], edge_d[t * P : t * P + P, lo:hi]
            )

        def dma_adj(t):
            nc.gpsimd.dma_start(adj_tiles[t][:], adj_r[:, t, :])

        # ---- ring: node W | t0 halves | t1..t4 full | 5a 6a 7a | 5b 6b 7b
        # node/W ride FIRST on the SWDGE wire (1MB, ~2.8us): the sync ring
        # starves behind the saturated SWDGE stream (W landed at ~45us when
        # loaded there, stalling every transpose-dependent op).
        nc.gpsimd.dma_start(node_all[:], node_d.rearrange("(t p) j -> p t j", p=P))
        nc.gpsimd.dma_start(w_all[:], w_d.rearrange("e i j -> i e j"))
        dma_edge(0, 0, H)
        dma_adj(0)
        dma_edge(0, H, N)

        # ident early on the GpSimd queue, wire already started
        ident = const_pool.tile([P, P], F32)
        make_identity(nc, ident[:])
        ident_bf = const_pool.tile([P, P], BF16)
        nc.vector.tensor_copy(ident_bf[:], ident[:])

        for t in range(1, 5):
            dma_adj(t)
            dma_edge(t, 0, N)
        for t in (5, 6, LAST):
            dma_adj(t)
            dma_edge(t, 0, H)
        dma_edge(5, H, N)
        dma_edge(6, H, N)
        dma_edge(LAST, H, N)

        # node^T / W^T via PE transpose -> bf16 (copies run on DVE in its
        # t0->t1 fill gap; emitted later so they can't HOL-block the STTs)
        nodeT = const_pool.tile([P, N], BF16)
        wT = const_pool.tile([P, E, D], BF16)  # [j, e, i]
        tcopies = []
        srcs = [(node_all[:, 0, :], nodeT[:, 0:P])]
        srcs += [(w_all[:, e, :], wT[:, e, :]) for e in range(E)]
        srcs += [(node_all[:, t, :], nodeT[:, bass.ts(t, P)]) for t in range(1, NT)]
        for src_ap, dst_ap in srcs:
            pt = pss_pool.tile([P, P], F32, tag="ps_small")
            nc.tensor.transpose(pt[:], src_ap, ident[:])
            tcopies.append((dst_ap, pt))

        coef5 = const_pool.tile([P, E], F32)
        coef6 = const_pool.tile([P, E], F32)
        coef7 = const_pool.tile([P, E], F32)
        v7sb = const_pool.tile([P, E, D], BF16)

        chunk_quads = {}

        def deint(t, lo, hi, planes, dve=False):
            """De-interleave of edge[t][:, lo:hi] into the quad layout
            (ScalarE, or DVE when it would otherwise idle at that point).
            Each chunk gets its own pool slot (lives only deint -> TTRs).
            planes=2 emits one op per int32-plane so the first 4 e-channels
            unblock after half the work."""
            quad = quad_pool.tile(
                [P, 2, N, 4], BF16, tag="quad", name=f"q{t}_{lo}"
            )
            chunk_quads[(t, lo)] = quad
            eng = nc.vector.tensor_copy if dve else nc.scalar.copy
            src = edge_tiles[t][:, lo:hi, :].bitcast(I32)
            for q in range(2) if planes == 2 else (None,):
                if q is None:
                    eng(
                        quad[:, :, lo:hi, :].bitcast(I32),
                        src.rearrange("p n (q t) -> p q n t", q=2),
                    )
                else:
                    eng(
                        quad[:, q, lo:hi, :].bitcast(I32),
                        src[:, :, 2 * q : 2 * q + 2],
                    )

        def ttr8(t, lo, hi, coef_dst, seed):
            """8 per-e STT mul-accumulates on DVE; seed=None starts the
            coef, otherwise a follow-up chunk lands in a temp slot that is
            merged with one [P,E] tensor_add."""
            quad = chunk_quads.pop((t, lo))
            scratch = work_pool.tile([P, N], BF16, tag="scratch")
            if seed is None:
                dst = coef_dst
            else:
                dst = work_pool.tile([P, E], F32, tag="coef_b", name=f"cb{t}{lo}")
            for e in range(E):
                q, j = divmod(e, 4)
                nc.vector.scalar_tensor_tensor(
                    out=scratch[:, 0 : hi - lo],
                    in0=quad[:, q, lo:hi, j],
                    scalar=1.0,
                    in1=adj_tiles[t][:, lo:hi],
                    op0=MUL,
                    op1=MUL,
                    accum_out=dst[:, e : e + 1],
                )
            if seed is not None:
                nc.vector.tensor_add(coef_dst[:], coef_dst[:], dst[:])

        def v_matmuls(t):
            psums = []
            for g in range(E // 4):
                pv = psv_pool.tile([P, 4, D], F32, tag="psum_v")
                nc.tensor.matmul(
                    pv[:],
                    lhsT=nodeT[:, bass.ts(t, P)],
                    rhs=wT[:, g * 4 : (g + 1) * 4, :],
                    start=True,
                    stop=True,
                )
                psums.append(pv)
            return psums

        state = {}

        def stage_reduce(t):
            """Sc scales + PE identity-matmul e-reduction + Sc out copy."""
            coef, psums = state.pop(t)
            sv = work_pool.tile([P, E, D], BF16, tag="sv")
            for e in range(E):
                nc.scalar.mul(
                    sv[:, e, :], psums[e // 4][:, e % 4, :], coef[:, e : e + 1]
                )
            acc = pss_pool.tile([P, D], F32, tag="ps_small")
            for e in range(E):
                nc.tensor.matmul(
                    acc[:],
                    lhsT=ident_bf[:],
                    rhs=sv[:, e, :],
                    start=(e == 0),
                    stop=(e == E - 1),
                )
            out_sb = work_pool.tile([P, D], F32, tag="out_sb")
            nc.scalar.copy(out_sb[:], acc[:])
            nc.sync.dma_start(out_d[bass.ts(t, P)], out_sb[:])

        # ---- software pipeline ----
        # t0 halves (deint is a single op per half: small chunks)
        coef0 = coef_pool.tile([P, E], F32, tag="coef")
        deint(0, 0, H, planes=1)
        ttr8(0, 0, H, coef0, seed=None)
        deint(0, H, N, planes=1)
        ttr8(0, H, N, coef0, seed=True)
        # DVE fill gap: the 16 transpose copies + park v7 in SBUF
        for dst_ap, pt in tcopies:
            nc.vector.tensor_copy(dst_ap, pt[:])
        psums7e = v_matmuls(LAST)
        for g in range(E // 4):
            nc.scalar.copy(v7sb[:, g * 4 : (g + 1) * 4, :], psums7e[g][:])
        state[0] = (coef0, v_matmuls(0))

        for t in range(1, 5):
            coef = coef_pool.tile([P, E], F32, tag="coef")
            deint(t, 0, N, planes=2)
            ttr8(t, 0, N, coef, seed=None)
            state[t] = (coef, v_matmuls(t))
            if t >= 2:
                stage_reduce(t - 2)

        # first halves of 5/6/7
        deint(5, 0, H, planes=1)
        ttr8(5, 0, H, coef5, seed=None)
        state[5] = (coef5, v_matmuls(5))
        deint(6, 0, H, planes=1)
        ttr8(6, 0, H, coef6, seed=None)
        state[6] = (coef6, v_matmuls(6))
        stage_reduce(3)
        deint(LAST, 0, H, planes=1)
        ttr8(LAST, 0, H, coef7, seed=None)

        # second halves; each seeds on the tile's existing coef
        deint(5, H, N, planes=1)
        ttr8(5, H, N, coef5, seed=True)
        stage_reduce(4)
        deint(6, H, N, planes=1)
        ttr8(6, H, N, coef6, seed=True)
        # final deint on DVE: it idles waiting for this data anyway, and
        # 1.2us there beats 2.3us+queueing on Sc
        deint(LAST, H, N, planes=1, dve=True)
        stage_reduce(5)
        ttr8(LAST, H, N, coef7, seed=True)
        stage_reduce(6)

        # t7 closeout: short DVE chain over v7 (SBUF reads)
        bufs = [
            work_pool.tile([P, D], F32, tag="acc_a", name="chain_a"),
            work_pool.tile([P, D], F32, tag="acc_b", name="chain_b"),
        ]
        nc.vector.tensor_scalar_mul(bufs[0][:], v7sb[:, 0, :], coef7[:, 0:1])
        cur_ap = bufs[0][:]
        for e in range(1, E):
            nxt = bufs[e % 2]
            nc.vector.scalar_tensor_tensor(
                out=nxt[:],
                in0=v7sb[:, e, :],
                scalar=coef7[:, e : e + 1],
                in1=cur_ap,
                op0=MUL,
                op1=ADD,
            )
            cur_ap = nxt[:]
        nc.sync.dma_start(out_d[bass.ts(LAST, P)], cur_ap)

    nc.compile()
    return nc


_NC_CACHE = None


def get_nc():
    global _NC_CACHE
    if _NC_CACHE is None:
        _NC_CACHE = build_nc()
    return _NC_CACHE


def make_in_maps(node_state, edge_type_mat, adj_mat, W):
    return [
        {
            "node_state": np.ascontiguousarray(node_state[b], dtype=np.float32),
            "edge_type_mat": np.ascontiguousarray(edge_type_mat[b], dtype=np.float32),
            "adj_mat": np.ascontiguousarray(adj_mat[b], dtype=np.float32),
            "W": np.ascontiguousarray(W, dtype=np.float32),
        }
        for b in range(B)
    ]


def kernel(node_state, edge_type_mat, adj_mat, W):
    nc = get_nc()
    in_maps = make_in_maps(node_state, edge_type_mat, adj_mat, W)
    res = run_bass_kernel_spmd(nc, in_maps, list(range(B)))
    return np.stack([res.results[b]["out"] for b in range(B)], axis=0)
